# revision 10
# baseline (speedup 1.0000x reference)
"""Trainium2 Bass kernel for nn_AttentionNet (audio-visual attention).

Data-parallel across 8 NeuronCores: batch B=256 split 32 per core, i.e.
n_n = 320 (b,t) rows and 320*49 = 15680 visual rows per core.

Per-core math (n indexes the 320 rows, s in [0,49), d/e in [0,512)):
    a_t = relu(audio @ Wa.T + ba)            [N,512]
    v_t = relu(vis @ Wv.T + bv)              [N,49,512]
    a_s = a_t @ Aa.T                         [N,49]
    v_s = v_t @ Av.T                         [N,49,49]
    f   = (tanh(a_s[:,:,None] + v_s)) @ Af.T [N,49]
    att = softmax_s(f)
    out = att @ vis                          [N,512]

Design notes:
- visual is pre-transposed ON THE HOST into a blocked layout
  visT[b, p, do, j] = vis[b*490 + j, do*128 + p] (fp16 for the epilogue
  weighted sum, fp8e4 copy for the Wv matmul), so the device never
  transposes the 32 MB visual tensor.
- The dominant Wv matmul runs in fp8e4 with DoubleRow perf mode
  (2 k-tiles of 128 per matmul).  Weights are pre-scaled by 32 on the
  host; the relu activation's free scale undoes it.
- The softmax runs unnormalized; 1/Z is folded into the final output
  transpose as a per-partition scale.
- The attention-weighted sum over s uses a masked tensor_tensor_scan
  (state = mask*state + t7) so segment sums appear at s=48 of each
  49-column group; fp16 intermediates keep the downcast noise at ~5e-4.
- The broadcast of the attention row across partitions and the Z
  (softmax denominator) path run on the otherwise-idle GPSIMD engine.
"""

import numpy as np

try:
    import concourse.bass as bass
except ImportError:
    import sys as _sys
    for _p in ("/opt/trn_rl_repo", "/root/.axon_site/_ro/trn_rl_repo"):
        if _p not in _sys.path:
            _sys.path.insert(0, _p)
    import concourse.bass as bass
import concourse.mybir as mybir
import concourse.tile as tile
from concourse import bacc
from concourse.bass import broadcast_tensor_aps

F32 = mybir.dt.float32
F16 = mybir.dt.float16
FP8 = mybir.dt.float8e4
AX = mybir.AxisListType
ALU = mybir.AluOpType
AF = mybir.ActivationFunctionType

NCORES = 8
B, T, S, D, E, A = 256, 10, 49, 512, 512, 128
NB = 10              # n's per column block
CB = NB * S          # 490 columns per block
NBLK = (B // NCORES) * T // NB   # 32 blocks per core

FP8_WV = True        # run the Wv matmul in fp8e4 with DoubleRow
WSCALE = 32.0        # host-side weight scale for fp8 dynamic range


def build_module(n_n, has_ba=False, has_bv=False):
    """Build the Bass module for one core handling n_n (b,t) rows."""
    assert n_n == NB * NBLK
    rows = n_n * S
    n_nt = (n_n + 127) // 128            # 128-row n tiles (a-path / epilogue)

    nc = bacc.Bacc("TRN2", debug=False)

    vist_d = nc.dram_tensor("visT", [NBLK, 128, 4, CB], F16,
                            kind="ExternalInput").ap()
    if FP8_WV:
        vis8_d = nc.dram_tensor("visT8", [NBLK, 128, 4, CB], FP8,
                                kind="ExternalInput").ap()
        wv8_d = nc.dram_tensor("Wv8", [128, 4, E], FP8,
                               kind="ExternalInput").ap()
    else:
        wvt_d = nc.dram_tensor("WvT", [128, 4, E], F16,
                               kind="ExternalInput").ap()
    aud_d = nc.dram_tensor("audio", [n_n, A], F16, kind="ExternalInput").ap()
    wat_d = nc.dram_tensor("WaT", [128, 4, 128], F16, kind="ExternalInput").ap()
    aat_d = nc.dram_tensor("AaT", [128, 4, S], F16, kind="ExternalInput").ap()
    avt_d = nc.dram_tensor("AvT", [128, 4, S], F16, kind="ExternalInput").ap()
    aft_d = nc.dram_tensor("AfT", [S, 1], F16, kind="ExternalInput").ap()
    bal_d = nc.dram_tensor("ba_l", [128, 4], F32, kind="ExternalInput").ap()
    bvl_d = nc.dram_tensor("bv_l", [128, 4], F32, kind="ExternalInput").ap()
    idn_d = nc.dram_tensor("ident", [128, 128], F16, kind="ExternalInput").ap()
    idf_d = nc.dram_tensor("identf", [128, 128], F32, kind="ExternalInput").ap()
    one_d = nc.dram_tensor("ones", [1, 128], F16, kind="ExternalInput").ap()
    msk_d = nc.dram_tensor("mask", [128, 4 * CB], F16, kind="ExternalInput").ap()
    out_d = nc.dram_tensor("out", [n_n, D], F32, kind="ExternalOutput").ap()

    with tile.TileContext(nc) as tc, \
         tc.tile_pool(name="consts", bufs=1) as cp, \
         tc.tile_pool(name="vload", bufs=4) as vp, \
         tc.tile_pool(name="work", bufs=2) as wp, \
         tc.tile_pool(name="dram", bufs=1, space="DRAM") as dp, \
         tc.tile_pool(name="ps_mm", bufs=2, space="PSUM") as pmm, \
         tc.tile_pool(name="ps_vs", bufs=2, space="PSUM") as pvs, \
         tc.tile_pool(name="ps_ft", bufs=2, space="PSUM") as pft:

        # ---------- constants ----------
        if FP8_WV:
            wv8 = cp.tile([128, 4, E], FP8, tag="wv8")
            nc.sync.dma_start(wv8[:], wv8_d)
        else:
            wvt = cp.tile([128, 4, E], F16, tag="wvt")
            nc.sync.dma_start(wvt[:], wvt_d)
        wat = cp.tile([128, 4, 128], F16, tag="wat")
        nc.sync.dma_start(wat[:], wat_d)
        aat = cp.tile([128, 4, S], F16, tag="aat")
        nc.sync.dma_start(aat[:], aat_d)
        avt = cp.tile([128, 4, S], F16, tag="avt")
        nc.sync.dma_start(avt[:], avt_d)
        aft = cp.tile([S, 1], F16, tag="aft")
        nc.sync.dma_start(aft[:], aft_d)
        ba = cp.tile([128, 4], F32, tag="ba")
        nc.sync.dma_start(ba[:], bal_d)
        bv = cp.tile([128, 4], F32, tag="bv")
        nc.sync.dma_start(bv[:], bvl_d)
        idn = cp.tile([128, 128], F16, tag="idn")
        nc.sync.dma_start(idn[:], idn_d)
        idf = cp.tile([128, 128], F32, tag="idf")
        nc.sync.dma_start(idf[:], idf_d)
        ones = cp.tile([1, 128], F16, tag="ones")
        nc.sync.dma_start(ones[:], one_d)
        mask = cp.tile([128, 4 * CB], F16, tag="mask")
        nc.sync.dma_start(mask[:], msk_d)

        audT = cp.tile([128, n_n], F16, tag="audT")     # audio.T  [a, n]
        atT = cp.tile([128, 4, n_n], F16, tag="atT")    # a_t.T    [e, n]
        asr = cp.tile([1, rows], F16, tag="asr")        # a_s row  [(n,s)]
        rinv = cp.tile([1, n_n], F32, tag="rinv")       # 1/Z per n
        outT = cp.tile([128, 4, n_n], F32, tag="outT")  # out.T    [d, n]

        # ---------- a-path prologue ----------
        for it in range(n_nt):
            n0 = it * 128
            nr = min(128, n_n - n0)
            an = wp.tile([128, A], F16, tag="an")
            nc.sync.dma_start(an[:nr, :], aud_d[n0:n0 + nr, :])
            ps = pvs.tile([128, 128], F16, tag="vs")
            nc.tensor.transpose(ps[:, :nr], an[:nr, :], idn[:nr, :nr])
            nc.vector.tensor_copy(audT[:, n0:n0 + nr], ps[:, :nr])

        for half in range(2):
            ps = pmm.tile([128, 2, 512], F32, tag="mm")
            for i in range(2):
                eo = half * 2 + i
                nc.tensor.matmul(ps[:, i, :n_n], wat[:, eo, :], audT[:],
                                 start=True, stop=True)
            if has_ba:
                for i in range(2):
                    eo = half * 2 + i
                    nc.scalar.activation(atT[:, eo, :], ps[:, i, :n_n],
                                         AF.Relu, bias=ba[:, eo:eo + 1])
            else:
                nc.scalar.activation(atT[:, half * 2:half * 2 + 2, :],
                                     ps[:, :, :n_n], AF.Relu)

        as_dram = dp.tile([1, rows], F16, tag="asd")
        for it in range(n_nt):
            n0 = it * 128
            nr = min(128, n_n - n0)
            psa = pvs.tile([128, 64], F32, tag="vs")
            for eo in range(4):
                nc.tensor.matmul(psa[:nr, :S], atT[:, eo, n0:n0 + nr],
                                 aat[:, eo, :],
                                 start=(eo == 0), stop=(eo == 3))
            asn = wp.tile([128, S], F16, tag="asn")
            nc.vector.tensor_copy(asn[:nr, :], psa[:nr, :S])
            dst = as_dram[0:1, n0 * S:(n0 + nr) * S]
            nc.sync.dma_start(dst.rearrange("one (n s) -> (one n) s", s=S),
                              asn[:nr, :])
        nc.sync.dma_start(asr[:], as_dram[:])

        # ---------- main loop ----------
        rscale = (1.0 / WSCALE) if FP8_WV else 1.0
        for b in range(NBLK):
            vis = vp.tile([128, 4, CB], F16, tag="vis")
            nc.sync.dma_start(vis[:], vist_d[b])
            if FP8_WV:
                vis8 = vp.tile([128, 4, CB], FP8, tag="vis8")
                nc.sync.dma_start(vis8[:], vis8_d[b])

            # v_t.T = relu(Wv @ vis.T + bv)   [e, col]
            vt = wp.tile([128, 4, CB], F16, tag="vt")
            for half in range(2):
                ps = pmm.tile([128, 2, 512], F32, tag="mm")
                for i in range(2):
                    eo = half * 2 + i
                    if FP8_WV:
                        for dp_ in range(2):
                            nc.tensor.matmul(
                                ps[:, i, :CB],
                                wv8[:, 2 * dp_:2 * dp_ + 2,
                                    eo * 128:(eo + 1) * 128],
                                vis8[:, 2 * dp_:2 * dp_ + 2, :],
                                start=(dp_ == 0), stop=(dp_ == 1),
                                perf_mode=mybir.MatmulPerfMode.DoubleRow)
                    else:
                        for do in range(4):
                            nc.tensor.matmul(
                                ps[:, i, :CB],
                                wvt[:, do, eo * 128:(eo + 1) * 128],
                                vis[:, do, :],
                                start=(do == 0), stop=(do == 3))
                # relu drain: ACT for eo 0,1,2 — DVE for eo 3 (load balance)
                if has_bv:
                    for i in range(2):
                        eo = half * 2 + i
                        if eo < 3:
                            nc.scalar.activation(vt[:, eo, :], ps[:, i, :CB],
                                                 AF.Relu, bias=bv[:, eo:eo + 1],
                                                 scale=rscale)
                        else:
                            # (rscale*x + bv) then relu, on DVE
                            nc.vector.tensor_scalar(vt[:, eo, :], ps[:, i, :CB],
                                                    rscale, None, ALU.mult)
                            nc.vector.tensor_scalar(vt[:, eo, :], vt[:, eo, :],
                                                    bv[:, eo:eo + 1], 0.0,
                                                    ALU.add, ALU.max)
                else:
                    if half == 0:
                        nc.scalar.activation(vt[:, 0:2, :],
                                             ps[:, :, :CB], AF.Relu,
                                             scale=rscale)
                    else:
                        nc.scalar.activation(vt[:, 2, :], ps[:, 0, :CB],
                                             AF.Relu, scale=rscale)
                        nc.vector.tensor_scalar(vt[:, 3, :], ps[:, 1, :CB],
                                                rscale, 0.0,
                                                ALU.mult, ALU.max)

            # v_s.T + a_s  [f, col]
            psv = pvs.tile([64, 512], F32, tag="vs")
            for eo in range(4):
                nc.tensor.matmul(psv[:S, :CB], avt[:, eo, :],
                                 vt[:, eo, :], start=(eo == 0), stop=False)
            nc.tensor.matmul(psv[:S, :CB], ones[0:1, 0:S],
                             asr[0:1, b * CB:(b + 1) * CB],
                             start=False, stop=True)

            th = wp.tile([S, CB], F16, tag="th")
            nc.scalar.activation(th[:], psv[:S, :CB], AF.Tanh)

            # f row = Af @ tanh  [1, col]
            psf = pft.tile([1, 512], F32, tag="ft")
            nc.tensor.matmul(psf[0:1, :CB], aft[:], th[:],
                             start=True, stop=True)

            # unnormalized softmax: e = exp(f); Z per n (normalize at end)
            ex = wp.tile([1, CB], F16, tag="ex")
            nc.scalar.activation(ex[:], psf[0:1, :CB], AF.Exp)

            # Z path: masked scan -> per-n sums at s=48
            zr = wp.tile([1, CB], F16, tag="zr")
            nc.vector.tensor_tensor_scan(zr[:], mask[0:1, 0:CB], ex[:],
                                         0.0, ALU.mult, ALU.add)
            zs = wp.tile([1, NB], F32, tag="zs")
            nc.vector.tensor_copy(
                zs[:], zr[:].rearrange("p (n s) -> p n s", s=S)[:, :, S - 1])
            nc.vector.reciprocal(rinv[0:1, b * NB:(b + 1) * NB], zs[:])

            # broadcast e across partitions on gpsimd
            ab = wp.tile([128, CB], F16, tag="ab")
            nc.gpsimd.partition_broadcast(ab[:], ex[:])

            # t7 = visT * att; segmented scan sums 49-col groups (s=48 slot)
            t7 = wp.tile([128, 4, CB], F16, tag="t7")
            in0, in1 = broadcast_tensor_aps(vis[:], ab[:].rearrange(
                "p (one c) -> p one c", one=1))
            nc.vector.tensor_tensor(t7[:], in0, in1, ALU.mult)
            z = wp.tile([128, 4 * CB], F16, tag="z")
            nc.vector.tensor_tensor_scan(z[:], mask[:],
                                         t7[:].rearrange("p f c -> p (f c)"),
                                         0.0, ALU.mult, ALU.add)
            nc.vector.tensor_copy(
                outT[:, :, b * NB:(b + 1) * NB],
                z[:].rearrange("p (f n s) -> p f n s", n=NB, s=S)[:, :, :, S - 1])

        # ---------- epilogue: transpose out.T back, scale by 1/Z, store ----------
        for it in range(n_nt):
            n0 = it * 128
            nr = min(128, n_n - n0)
            psr = pvs.tile([128, 128], F32, tag="vs")
            nc.tensor.transpose(psr[:nr, 0:1], rinv[0:1, n0:n0 + nr],
                                idf[0:1, 0:1])
            rin = wp.tile([128, 1], F32, tag="rin")
            nc.vector.tensor_copy(rin[:nr, :], psr[:nr, 0:1])
            on = wp.tile([128, D], F32, tag="on")
            for do in range(4):
                pso = pmm.tile([128, 2, 512], F32, tag="mm")
                nc.tensor.transpose(pso[:nr, 0, :128], outT[:, do, n0:n0 + nr],
                                    idf[:, :])
                nc.scalar.activation(on[:nr, do * 128:(do + 1) * 128],
                                     pso[:nr, 0, :128], AF.Copy,
                                     scale=rin[:nr, 0:1])
            nc.sync.dma_start(out_d[n0:n0 + nr, :], on[:nr, :])

    nc.finalize()
    return nc


def prep_consts(Wa, ba, Wv, bv, Aa, Av, Af):
    import ml_dtypes
    f16 = np.float16
    f = np.float32
    c = {}
    # wvt[p, do, e] = Wv[e, do*128+p]
    wvt_host = np.ascontiguousarray(Wv.T.reshape(4, 128, E).transpose(1, 0, 2))
    if FP8_WV:
        c["Wv8"] = (wvt_host * WSCALE).astype(ml_dtypes.float8_e4m3)
    else:
        c["WvT"] = wvt_host.astype(f16)
    # wat[a, eo, m] = Wa[eo*128+m, a]
    c["WaT"] = np.ascontiguousarray(
        Wa.T.reshape(A, 4, 128)).astype(f16)
    # aat[p, eo, s] = Aa[s, eo*128+p]
    c["AaT"] = np.ascontiguousarray(
        Aa.T.reshape(4, 128, S).transpose(1, 0, 2)).astype(f16)
    c["AvT"] = np.ascontiguousarray(
        Av.T.reshape(4, 128, S).transpose(1, 0, 2)).astype(f16)
    c["AfT"] = np.ascontiguousarray(Af.reshape(1, S).T).astype(f16)
    c["ba_l"] = np.ascontiguousarray(ba.reshape(4, 128).T).astype(f)
    c["bv_l"] = np.ascontiguousarray(bv.reshape(4, 128).T).astype(f)
    c["ident"] = np.eye(128, dtype=f).astype(f16)
    c["identf"] = np.eye(128, dtype=f)
    c["ones"] = np.ones((1, 128), dtype=f16)
    # mask: 0 at the start of each 49-column segment, 1 elsewhere
    msk = np.ones((128, 4 * CB), dtype=f16)
    msk[:, ::S] = 0.0
    c["mask"] = msk
    return c


_CACHE = {}


def kernel(audio, visual, Wa, ba, Wv, bv, Aa, Av, Af):
    from concourse.bass_utils import run_bass_kernel_spmd
    import ml_dtypes
    f16 = np.float16

    audio = np.asarray(audio, np.float32)
    visual = np.asarray(visual, np.float32)
    ba = np.asarray(ba, np.float32)
    bv = np.asarray(bv, np.float32)
    n_n = (B // NCORES) * T  # 320

    has_ba = bool(np.any(ba))
    has_bv = bool(np.any(bv))
    key = ("nc", has_ba, has_bv)
    if key not in _CACHE:
        _CACHE[key] = build_module(n_n, has_ba, has_bv)
    nc = _CACHE[key]

    consts = prep_consts(np.asarray(Wa, np.float32), ba,
                         np.asarray(Wv, np.float32), bv,
                         np.asarray(Aa, np.float32),
                         np.asarray(Av, np.float32),
                         np.asarray(Af, np.float32))
    bs = B // NCORES
    vis_16 = visual.astype(f16)          # one cast for the full tensor
    aud_16 = audio.astype(f16)
    in_maps = []
    for c in range(NCORES):
        m = dict(consts)
        m["audio"] = np.ascontiguousarray(
            aud_16[c * bs:(c + 1) * bs].reshape(n_n, A))
        # visT[b, p, do, j] = vis[b*CB + j, do*128 + p]
        v = vis_16[c * bs:(c + 1) * bs].reshape(NBLK, CB, 4, 128)
        vT = np.ascontiguousarray(v.transpose(0, 3, 2, 1))
        m["visT"] = vT
        if FP8_WV:
            m["visT8"] = vT.astype(ml_dtypes.float8_e4m3)
        in_maps.append(m)

    res = run_bass_kernel_spmd(nc, in_maps, core_ids=list(range(NCORES)))
    _CACHE["last_res"] = res
    out = np.concatenate(
        [r["out"].reshape(bs, T, D) for r in res.results], axis=0)
    return out.astype(np.float32)


# revision 18
# speedup vs baseline: 1.0654x; 1.0654x over previous
"""Trainium2 Bass kernel for nn_AttentionNet (audio-visual attention).

Data-parallel across 8 NeuronCores: batch B=256 split 32 per core, i.e.
n_n = 320 (b,t) rows and 320*49 = 15680 visual rows per core.

Per-core math (n indexes the 320 rows, s in [0,49), d/e in [0,512)):
    a_t = relu(audio @ Wa.T + ba)            [N,512]
    v_t = relu(vis @ Wv.T + bv)              [N,49,512]
    a_s = a_t @ Aa.T                         [N,49]
    v_s = v_t @ Av.T                         [N,49,49]
    f   = (tanh(a_s[:,:,None] + v_s)) @ Af.T [N,49]
    att = softmax_s(f)
    out = att @ vis                          [N,512]

Design notes:
- visual is pre-transposed ON THE HOST into a blocked layout
  visT[b, p, do, j] = vis[b*490 + j, do*128 + p] (fp16 for the epilogue
  weighted sum, fp8e4 copy for the Wv matmul), so the device never
  transposes the 32 MB visual tensor.
- The dominant Wv matmul runs in fp8e4 with DoubleRow perf mode
  (2 k-tiles of 128 per matmul).  Weights are pre-scaled by 32 on the
  host; the relu activation's free scale undoes it.
- The softmax runs unnormalized; 1/Z is folded into the final output
  transpose as a per-partition scale.
- The attention-weighted sum over s uses a masked tensor_tensor_scan
  (state = mask*state + t7) so segment sums appear at s=48 of each
  49-column group; fp16 intermediates keep the downcast noise at ~5e-4.
- The broadcast of the attention row across partitions and the Z
  (softmax denominator) path run on the otherwise-idle GPSIMD engine.
"""

import numpy as np

try:
    import concourse.bass as bass
except ImportError:
    import sys as _sys
    for _p in ("/opt/trn_rl_repo", "/root/.axon_site/_ro/trn_rl_repo"):
        if _p not in _sys.path:
            _sys.path.insert(0, _p)
    import concourse.bass as bass
import concourse.mybir as mybir
import concourse.tile as tile
from concourse import bacc
from concourse.bass import broadcast_tensor_aps

F32 = mybir.dt.float32
F16 = mybir.dt.float16
FP8 = mybir.dt.float8e4
AX = mybir.AxisListType
ALU = mybir.AluOpType
AF = mybir.ActivationFunctionType

NCORES = 8
B, T, S, D, E, A = 256, 10, 49, 512, 512, 128
NB = 10              # n's per column block
CB = NB * S          # 490 columns per block
NBLK = (B // NCORES) * T // NB   # 32 blocks per core

FP8_WV = True        # run the Wv matmul in fp8e4 with DoubleRow
WSCALE = 32.0        # host-side weight scale for fp8 dynamic range


def build_module(n_n, has_ba=False, has_bv=False):
    """Build the Bass module for one core handling n_n (b,t) rows."""
    assert n_n == NB * NBLK
    rows = n_n * S
    n_nt = (n_n + 127) // 128            # 128-row n tiles (a-path / epilogue)

    nc = bacc.Bacc("TRN2", debug=False)

    vist_d = nc.dram_tensor("visT", [NBLK, 128, 4, CB], F16,
                            kind="ExternalInput").ap()
    if FP8_WV:
        vis8_d = nc.dram_tensor("visT8", [NBLK, 128, 4, CB], FP8,
                                kind="ExternalInput").ap()
        wv8_d = nc.dram_tensor("Wv8", [128, 4, E], FP8,
                               kind="ExternalInput").ap()
    else:
        wvt_d = nc.dram_tensor("WvT", [128, 4, E], F16,
                               kind="ExternalInput").ap()
    aud_d = nc.dram_tensor("audio", [n_n, A], F16, kind="ExternalInput").ap()
    wat_d = nc.dram_tensor("WaT", [128, 4, 128], F16, kind="ExternalInput").ap()
    aat_d = nc.dram_tensor("AaT", [128, 4, S], F16, kind="ExternalInput").ap()
    avt_d = nc.dram_tensor("AvT", [128, 4, S], F16, kind="ExternalInput").ap()
    aft_d = nc.dram_tensor("AfT", [S, 1], F16, kind="ExternalInput").ap()
    bal_d = nc.dram_tensor("ba_l", [128, 4], F32, kind="ExternalInput").ap()
    bvl_d = nc.dram_tensor("bv_l", [128, 4], F32, kind="ExternalInput").ap()
    idn_d = nc.dram_tensor("ident", [128, 128], F16, kind="ExternalInput").ap()
    idf_d = nc.dram_tensor("identf", [128, 128], F32, kind="ExternalInput").ap()
    one_d = nc.dram_tensor("ones", [1, 128], F16, kind="ExternalInput").ap()
    out_d = nc.dram_tensor("out", [n_n, D], F32, kind="ExternalOutput").ap()

    with tile.TileContext(nc) as tc, \
         tc.tile_pool(name="consts", bufs=1) as cp, \
         tc.tile_pool(name="vload", bufs=4) as vp, \
         tc.tile_pool(name="work", bufs=2) as wp, \
         tc.tile_pool(name="dram", bufs=1, space="DRAM") as dp, \
         tc.tile_pool(name="ps_mm", bufs=2, space="PSUM") as pmm, \
         tc.tile_pool(name="ps_vs", bufs=2, space="PSUM") as pvs, \
         tc.tile_pool(name="ps_ft", bufs=2, space="PSUM") as pft:

        # ---------- constants ----------
        if FP8_WV:
            wv8 = cp.tile([128, 4, E], FP8, tag="wv8")
            nc.sync.dma_start(wv8[:], wv8_d)
        else:
            wvt = cp.tile([128, 4, E], F16, tag="wvt")
            nc.sync.dma_start(wvt[:], wvt_d)
        wat = cp.tile([128, 4, 128], F16, tag="wat")
        nc.sync.dma_start(wat[:], wat_d)
        aat = cp.tile([128, 4, S], F16, tag="aat")
        nc.sync.dma_start(aat[:], aat_d)
        avt = cp.tile([128, 4, S], F16, tag="avt")
        nc.sync.dma_start(avt[:], avt_d)
        aft = cp.tile([S, 1], F16, tag="aft")
        nc.sync.dma_start(aft[:], aft_d)
        ba = cp.tile([128, 4], F32, tag="ba")
        nc.sync.dma_start(ba[:], bal_d)
        bv = cp.tile([128, 4], F32, tag="bv")
        nc.sync.dma_start(bv[:], bvl_d)
        idn = cp.tile([128, 128], F16, tag="idn")
        nc.sync.dma_start(idn[:], idn_d)
        idf = cp.tile([128, 128], F32, tag="idf")
        nc.sync.dma_start(idf[:], idf_d)
        ones = cp.tile([1, 128], F16, tag="ones")
        nc.sync.dma_start(ones[:], one_d)

        audT = cp.tile([128, n_n], F16, tag="audT")     # audio.T  [a, n]
        atT = cp.tile([128, 4, n_n], F16, tag="atT")    # a_t.T    [e, n]
        asr = cp.tile([1, rows], F16, tag="asr")        # a_s row  [(n,s)]
        rinv = cp.tile([1, n_n], F32, tag="rinv")       # 1/Z per n
        # out.T [d, n] in chunks 0..3; chunk 4 row 0 = Z per n
        outT = cp.tile([128, 5, n_n], F32, tag="outT")

        # ---------- a-path prologue ----------
        for it in range(n_nt):
            n0 = it * 128
            nr = min(128, n_n - n0)
            an = wp.tile([128, A], F16, tag="an")
            nc.sync.dma_start(an[:nr, :], aud_d[n0:n0 + nr, :])
            ps = pvs.tile([128, 128], F16, tag="vs")
            nc.tensor.transpose(ps[:, :nr], an[:nr, :], idn[:nr, :nr])
            nc.vector.tensor_copy(audT[:, n0:n0 + nr], ps[:, :nr])

        for half in range(2):
            ps = pmm.tile([128, 2, 512], F32, tag="mm")
            for i in range(2):
                eo = half * 2 + i
                nc.tensor.matmul(ps[:, i, :n_n], wat[:, eo, :], audT[:],
                                 start=True, stop=True)
            if has_ba:
                for i in range(2):
                    eo = half * 2 + i
                    nc.scalar.activation(atT[:, eo, :], ps[:, i, :n_n],
                                         AF.Relu, bias=ba[:, eo:eo + 1])
            else:
                nc.scalar.activation(atT[:, half * 2:half * 2 + 2, :],
                                     ps[:, :, :n_n], AF.Relu)

        as_dram = dp.tile([1, rows], F16, tag="asd")
        for it in range(n_nt):
            n0 = it * 128
            nr = min(128, n_n - n0)
            psa = pvs.tile([128, 64], F32, tag="vs")
            for eo in range(4):
                nc.tensor.matmul(psa[:nr, :S], atT[:, eo, n0:n0 + nr],
                                 aat[:, eo, :],
                                 start=(eo == 0), stop=(eo == 3))
            asn = wp.tile([128, S], F16, tag="asn")
            nc.vector.tensor_copy(asn[:nr, :], psa[:nr, :S])
            dst = as_dram[0:1, n0 * S:(n0 + nr) * S]
            nc.sync.dma_start(dst.rearrange("one (n s) -> (one n) s", s=S),
                              asn[:nr, :])
        nc.sync.dma_start(asr[:], as_dram[:])

        # ---------- main loop ----------
        rscale = (1.0 / WSCALE) if FP8_WV else 1.0
        for b in range(NBLK):
            vis = vp.tile([128, 4, CB], F16, tag="vis", bufs=6)
            nc.sync.dma_start(vis[:], vist_d[b])
            if FP8_WV:
                vis8 = vp.tile([128, 4, CB], FP8, tag="vis8", bufs=6)
                nc.scalar.dma_start(vis8[:], vis8_d[b])

            # v_t.T = relu(Wv @ vis.T + bv)   [e, col]
            vt = wp.tile([128, 4, CB], F16, tag="vt", bufs=3)
            for half in range(2):
                ps = pmm.tile([128, 2, 512], F32, tag="mm")
                for i in range(2):
                    eo = half * 2 + i
                    if FP8_WV:
                        for dp_ in range(2):
                            nc.tensor.matmul(
                                ps[:, i, :CB],
                                wv8[:, 2 * dp_:2 * dp_ + 2,
                                    eo * 128:(eo + 1) * 128],
                                vis8[:, 2 * dp_:2 * dp_ + 2, :],
                                start=(dp_ == 0), stop=(dp_ == 1),
                                perf_mode=mybir.MatmulPerfMode.DoubleRow)
                    else:
                        for do in range(4):
                            nc.tensor.matmul(
                                ps[:, i, :CB],
                                wvt[:, do, eo * 128:(eo + 1) * 128],
                                vis[:, do, :],
                                start=(do == 0), stop=(do == 3))
                # relu drain: ACT for eo 0,1,2 — DVE for eo 3 (load balance)
                if has_bv:
                    for i in range(2):
                        eo = half * 2 + i
                        if eo < 3:
                            nc.scalar.activation(vt[:, eo, :], ps[:, i, :CB],
                                                 AF.Relu, bias=bv[:, eo:eo + 1],
                                                 scale=rscale)
                        else:
                            # (rscale*x + bv) then relu, on DVE
                            nc.vector.tensor_scalar(vt[:, eo, :], ps[:, i, :CB],
                                                    rscale, None, ALU.mult)
                            nc.vector.tensor_scalar(vt[:, eo, :], vt[:, eo, :],
                                                    bv[:, eo:eo + 1], 0.0,
                                                    ALU.add, ALU.max)
                else:
                    if half == 0:
                        nc.scalar.activation(vt[:, 0:2, :],
                                             ps[:, :, :CB], AF.Relu,
                                             scale=rscale)
                    else:
                        nc.scalar.activation(vt[:, 2, :], ps[:, 0, :CB],
                                             AF.Relu, scale=rscale)
                        nc.vector.tensor_scalar(vt[:, 3, :], ps[:, 1, :CB],
                                                rscale, 0.0,
                                                ALU.mult, ALU.max)

            # v_s.T + a_s  [f, col]
            psv = pvs.tile([64, 512], F32, tag="vs")
            for eo in range(4):
                nc.tensor.matmul(psv[:S, :CB], avt[:, eo, :],
                                 vt[:, eo, :], start=(eo == 0), stop=False)
            nc.tensor.matmul(psv[:S, :CB], ones[0:1, 0:S],
                             asr[0:1, b * CB:(b + 1) * CB],
                             start=False, stop=True)

            th = wp.tile([S, CB], F16, tag="th", bufs=3)
            nc.scalar.activation(th[:], psv[:S, :CB], AF.Tanh)

            # f row = Af @ tanh  [1, col]
            psf = pft.tile([1, 512], F32, tag="ft")
            nc.tensor.matmul(psf[0:1, :CB], aft[:], th[:],
                             start=True, stop=True)

            # unnormalized softmax: e = exp(f); Z per n (normalize at end)
            ex = wp.tile([1, CB], F16, tag="ex", bufs=3)
            nc.scalar.activation(ex[:], psf[0:1, :CB], AF.Exp)

            # broadcast e across partitions into t7 chunk 4 (gpsimd)
            t7 = wp.tile([128, 5, CB], F16, tag="t7", bufs=3)
            nc.gpsimd.partition_broadcast(t7[:, 4, :], ex[:])

            # t7[0:4] = visT * att (split between DVE and gpsimd)
            in0, in1 = broadcast_tensor_aps(vis[:, 0:2, :], t7[:, 4:5, :])
            nc.vector.tensor_tensor(t7[:, 0:2, :], in0, in1, ALU.mult)
            in0, in1 = broadcast_tensor_aps(vis[:, 2:4, :], t7[:, 4:5, :])
            nc.gpsimd.tensor_tensor(t7[:, 2:4, :], in0, in1, ALU.mult)

            # segmented sums over s for all 5 chunks on DVE;
            # outT chunk 4 = Z replicated per partition.
            nc.vector.reduce_sum(
                outT[:, :, b * NB:(b + 1) * NB],
                t7[:].rearrange("p f (n s) -> p f n s", s=S),
                axis=AX.X)
            nc.vector.reciprocal(rinv[0:1, b * NB:(b + 1) * NB],
                                 outT[0:1, 4, b * NB:(b + 1) * NB])

        # ---------- epilogue: transpose out.T back, scale by 1/Z, store ----------
        for it in range(n_nt):
            n0 = it * 128
            nr = min(128, n_n - n0)
            psr = pvs.tile([128, 128], F32, tag="vs")
            nc.tensor.transpose(psr[:nr, 0:1], rinv[0:1, n0:n0 + nr],
                                idf[0:1, 0:1])
            rin = wp.tile([128, 1], F32, tag="rin")
            nc.vector.tensor_copy(rin[:nr, :], psr[:nr, 0:1])
            on = wp.tile([128, D], F32, tag="on")
            for do in range(4):
                pso = pmm.tile([128, 2, 512], F32, tag="mm")
                nc.tensor.transpose(pso[:nr, 0, :128], outT[:, do, n0:n0 + nr],
                                    idf[:, :])
                nc.scalar.activation(on[:nr, do * 128:(do + 1) * 128],
                                     pso[:nr, 0, :128], AF.Copy,
                                     scale=rin[:nr, 0:1])
            nc.sync.dma_start(out_d[n0:n0 + nr, :], on[:nr, :])

    nc.finalize()
    return nc


def prep_consts(Wa, ba, Wv, bv, Aa, Av, Af):
    import ml_dtypes
    f16 = np.float16
    f = np.float32
    c = {}
    # wvt[p, do, e] = Wv[e, do*128+p]
    wvt_host = np.ascontiguousarray(Wv.T.reshape(4, 128, E).transpose(1, 0, 2))
    if FP8_WV:
        c["Wv8"] = (wvt_host * WSCALE).astype(ml_dtypes.float8_e4m3)
    else:
        c["WvT"] = wvt_host.astype(f16)
    # wat[a, eo, m] = Wa[eo*128+m, a]
    c["WaT"] = np.ascontiguousarray(
        Wa.T.reshape(A, 4, 128)).astype(f16)
    # aat[p, eo, s] = Aa[s, eo*128+p]
    c["AaT"] = np.ascontiguousarray(
        Aa.T.reshape(4, 128, S).transpose(1, 0, 2)).astype(f16)
    c["AvT"] = np.ascontiguousarray(
        Av.T.reshape(4, 128, S).transpose(1, 0, 2)).astype(f16)
    c["AfT"] = np.ascontiguousarray(Af.reshape(1, S).T).astype(f16)
    c["ba_l"] = np.ascontiguousarray(ba.reshape(4, 128).T).astype(f)
    c["bv_l"] = np.ascontiguousarray(bv.reshape(4, 128).T).astype(f)
    c["ident"] = np.eye(128, dtype=f).astype(f16)
    c["identf"] = np.eye(128, dtype=f)
    c["ones"] = np.ones((1, 128), dtype=f16)
    return c


_CACHE = {}


def kernel(audio, visual, Wa, ba, Wv, bv, Aa, Av, Af):
    from concourse.bass_utils import run_bass_kernel_spmd
    import ml_dtypes
    f16 = np.float16

    audio = np.asarray(audio, np.float32)
    visual = np.asarray(visual, np.float32)
    ba = np.asarray(ba, np.float32)
    bv = np.asarray(bv, np.float32)
    n_n = (B // NCORES) * T  # 320

    has_ba = bool(np.any(ba))
    has_bv = bool(np.any(bv))
    key = ("nc", has_ba, has_bv)
    if key not in _CACHE:
        _CACHE[key] = build_module(n_n, has_ba, has_bv)
    nc = _CACHE[key]

    consts = prep_consts(np.asarray(Wa, np.float32), ba,
                         np.asarray(Wv, np.float32), bv,
                         np.asarray(Aa, np.float32),
                         np.asarray(Av, np.float32),
                         np.asarray(Af, np.float32))
    bs = B // NCORES
    vis_16 = visual.astype(f16)          # one cast for the full tensor
    aud_16 = audio.astype(f16)
    in_maps = []
    for c in range(NCORES):
        m = dict(consts)
        m["audio"] = np.ascontiguousarray(
            aud_16[c * bs:(c + 1) * bs].reshape(n_n, A))
        # visT[b, p, do, j] = vis[b*CB + j, do*128 + p]
        v = vis_16[c * bs:(c + 1) * bs].reshape(NBLK, CB, 4, 128)
        vT = np.ascontiguousarray(v.transpose(0, 3, 2, 1))
        m["visT"] = vT
        if FP8_WV:
            m["visT8"] = vT.astype(ml_dtypes.float8_e4m3)
        in_maps.append(m)

    res = run_bass_kernel_spmd(nc, in_maps, core_ids=list(range(NCORES)))
    _CACHE["last_res"] = res
    out = np.concatenate(
        [r["out"].reshape(bs, T, D) for r in res.results], axis=0)
    return out.astype(np.float32)


# revision 19
# speedup vs baseline: 1.3981x; 1.3123x over previous
"""Trainium2 Bass kernel for nn_AttentionNet (audio-visual attention).

Data-parallel across 8 NeuronCores: batch B=256 split 32 per core, i.e.
n_n = 320 (b,t) rows and 320*49 = 15680 visual rows per core.

Per-core math (n indexes the 320 rows, s in [0,49), d/e in [0,512)):
    a_t = relu(audio @ Wa.T + ba)            [N,512]
    v_t = relu(vis @ Wv.T + bv)              [N,49,512]
    a_s = a_t @ Aa.T                         [N,49]
    v_s = v_t @ Av.T                         [N,49,49]
    f   = (tanh(a_s[:,:,None] + v_s)) @ Af.T [N,49]
    att = softmax_s(f)
    out = att @ vis                          [N,512]

Design notes:
- visual is pre-transposed ON THE HOST into a blocked layout
  visT[b, p, do, j] = vis[b*490 + j, do*128 + p] (fp16 for the epilogue
  weighted sum, fp8e4 copy for the Wv matmul), so the device never
  transposes the 32 MB visual tensor.
- The dominant Wv matmul runs in fp8e4 with DoubleRow perf mode
  (2 k-tiles of 128 per matmul).  Weights are pre-scaled by 32 on the
  host; the relu activation's free scale undoes it.
- The softmax runs unnormalized; 1/Z is folded into the final output
  transpose as a per-partition scale.
- The attention-weighted sum over s uses a masked tensor_tensor_scan
  (state = mask*state + t7) so segment sums appear at s=48 of each
  49-column group; fp16 intermediates keep the downcast noise at ~5e-4.
- The broadcast of the attention row across partitions and the Z
  (softmax denominator) path run on the otherwise-idle GPSIMD engine.
"""

import numpy as np

try:
    import concourse.bass as bass
except ImportError:
    import sys as _sys
    for _p in ("/opt/trn_rl_repo", "/root/.axon_site/_ro/trn_rl_repo"):
        if _p not in _sys.path:
            _sys.path.insert(0, _p)
    import concourse.bass as bass
import concourse.mybir as mybir
import concourse.tile as tile
from concourse import bacc
from concourse.bass import broadcast_tensor_aps

F32 = mybir.dt.float32
F16 = mybir.dt.float16
FP8 = mybir.dt.float8e4
AX = mybir.AxisListType
ALU = mybir.AluOpType
AF = mybir.ActivationFunctionType

NCORES = 8
B, T, S, D, E, A = 256, 10, 49, 512, 512, 128
NB = 10              # n's per column block
CB = NB * S          # 490 columns per block
NBLK = (B // NCORES) * T // NB   # 32 blocks per core

FP8_WV = True        # run the Wv matmul in fp8e4 with DoubleRow
WSCALE = 32.0        # host-side weight scale for fp8 dynamic range


def build_module(n_n, has_ba=False, has_bv=False):
    """Build the Bass module for one core handling n_n (b,t) rows."""
    assert n_n == NB * NBLK
    rows = n_n * S
    n_nt = (n_n + 127) // 128            # 128-row n tiles (a-path / epilogue)

    nc = bacc.Bacc("TRN2", debug=False)

    vist_d = nc.dram_tensor("visT", [NBLK, 128, 4, CB], F16,
                            kind="ExternalInput").ap()
    if FP8_WV:
        vis8_d = nc.dram_tensor("visT8", [NBLK, 128, 4, CB], FP8,
                                kind="ExternalInput").ap()
        wv8_d = nc.dram_tensor("Wv8", [128, 4, E], FP8,
                               kind="ExternalInput").ap()
    else:
        wvt_d = nc.dram_tensor("WvT", [128, 4, E], F16,
                               kind="ExternalInput").ap()
    aud_d = nc.dram_tensor("audio", [n_n, A], F16, kind="ExternalInput").ap()
    wat_d = nc.dram_tensor("WaT", [128, 4, 128], F16, kind="ExternalInput").ap()
    aat_d = nc.dram_tensor("AaT", [128, 4, S], F16, kind="ExternalInput").ap()
    avt_d = nc.dram_tensor("AvT", [128, 4, S], F16, kind="ExternalInput").ap()
    aft_d = nc.dram_tensor("AfT", [S, 1], F16, kind="ExternalInput").ap()
    bal_d = nc.dram_tensor("ba_l", [128, 4], F32, kind="ExternalInput").ap()
    bvl_d = nc.dram_tensor("bv_l", [128, 4], F32, kind="ExternalInput").ap()
    idn_d = nc.dram_tensor("ident", [128, 128], F16, kind="ExternalInput").ap()
    idf_d = nc.dram_tensor("identf", [128, 128], F32, kind="ExternalInput").ap()
    one_d = nc.dram_tensor("ones", [1, 128], F16, kind="ExternalInput").ap()
    out_d = nc.dram_tensor("out", [n_n, D], F32, kind="ExternalOutput").ap()

    with tile.TileContext(nc) as tc, \
         tc.tile_pool(name="consts", bufs=1) as cp, \
         tc.tile_pool(name="vload", bufs=4) as vp, \
         tc.tile_pool(name="work", bufs=2) as wp, \
         tc.tile_pool(name="dram", bufs=1, space="DRAM") as dp, \
         tc.tile_pool(name="ps_mm", bufs=2, space="PSUM") as pmm, \
         tc.tile_pool(name="ps_vs", bufs=2, space="PSUM") as pvs, \
         tc.tile_pool(name="ps_ft", bufs=2, space="PSUM") as pft:

        # ---------- constants ----------
        if FP8_WV:
            wv8 = cp.tile([128, 4, E], FP8, tag="wv8")
            nc.sync.dma_start(wv8[:], wv8_d)
        else:
            wvt = cp.tile([128, 4, E], F16, tag="wvt")
            nc.sync.dma_start(wvt[:], wvt_d)
        wat = cp.tile([128, 4, 128], F16, tag="wat")
        nc.sync.dma_start(wat[:], wat_d)
        aat = cp.tile([128, 4, S], F16, tag="aat")
        nc.sync.dma_start(aat[:], aat_d)
        avt = cp.tile([128, 4, S], F16, tag="avt")
        nc.sync.dma_start(avt[:], avt_d)
        aft = cp.tile([S, 1], F16, tag="aft")
        nc.sync.dma_start(aft[:], aft_d)
        ba = cp.tile([128, 4], F32, tag="ba")
        nc.sync.dma_start(ba[:], bal_d)
        bv = cp.tile([128, 4], F32, tag="bv")
        nc.sync.dma_start(bv[:], bvl_d)
        idn = cp.tile([128, 128], F16, tag="idn")
        nc.sync.dma_start(idn[:], idn_d)
        idf = cp.tile([128, 128], F32, tag="idf")
        nc.sync.dma_start(idf[:], idf_d)
        ones = cp.tile([1, 128], F16, tag="ones")
        nc.sync.dma_start(ones[:], one_d)

        audT = cp.tile([128, n_n], F16, tag="audT")     # audio.T  [a, n]
        atT = cp.tile([128, 4, n_n], F16, tag="atT")    # a_t.T    [e, n]
        asr = cp.tile([1, rows], F16, tag="asr")        # a_s row  [(n,s)]
        rinv = cp.tile([1, n_n], F32, tag="rinv")       # 1/Z per n
        # out.T [d, n] in chunks 0..3; chunk 4 row 0 = Z per n
        outT = cp.tile([128, 5, n_n], F32, tag="outT")

        # ---------- a-path prologue ----------
        for it in range(n_nt):
            n0 = it * 128
            nr = min(128, n_n - n0)
            an = wp.tile([128, A], F16, tag="an")
            nc.sync.dma_start(an[:nr, :], aud_d[n0:n0 + nr, :])
            ps = pvs.tile([128, 128], F16, tag="vs")
            nc.tensor.transpose(ps[:, :nr], an[:nr, :], idn[:nr, :nr])
            nc.vector.tensor_copy(audT[:, n0:n0 + nr], ps[:, :nr])

        for half in range(2):
            ps = pmm.tile([128, 2, 512], F32, tag="mm")
            for i in range(2):
                eo = half * 2 + i
                nc.tensor.matmul(ps[:, i, :n_n], wat[:, eo, :], audT[:],
                                 start=True, stop=True)
            if has_ba:
                for i in range(2):
                    eo = half * 2 + i
                    nc.scalar.activation(atT[:, eo, :], ps[:, i, :n_n],
                                         AF.Relu, bias=ba[:, eo:eo + 1])
            else:
                nc.scalar.activation(atT[:, half * 2:half * 2 + 2, :],
                                     ps[:, :, :n_n], AF.Relu)

        as_dram = dp.tile([1, rows], F16, tag="asd")
        for it in range(n_nt):
            n0 = it * 128
            nr = min(128, n_n - n0)
            psa = pvs.tile([128, 64], F32, tag="vs")
            for eo in range(4):
                nc.tensor.matmul(psa[:nr, :S], atT[:, eo, n0:n0 + nr],
                                 aat[:, eo, :],
                                 start=(eo == 0), stop=(eo == 3))
            asn = wp.tile([128, S], F16, tag="asn")
            nc.vector.tensor_copy(asn[:nr, :], psa[:nr, :S])
            dst = as_dram[0:1, n0 * S:(n0 + nr) * S]
            nc.sync.dma_start(dst.rearrange("one (n s) -> (one n) s", s=S),
                              asn[:nr, :])
        nc.sync.dma_start(asr[:], as_dram[:])

        # ---------- main loop (software-pipelined, 2-block skew) ----------
        # stage A(b): DMA + Wv matmul + relu          -> vt[b]
        # stage B(b): v_s matmul + a_s add + tanh     -> th[b]
        # stage C(b): f matmul, exp, broadcast, att*vis, segment-reduce
        # Consecutive instructions on each engine depend only on work from
        # previous blocks, so no engine head-of-line blocks on same-block
        # results.
        rscale = (1.0 / WSCALE) if FP8_WV else 1.0
        visq, vtq, thq = {}, {}, {}

        def stage_a(b):
            vis = vp.tile([128, 4, CB], F16, tag="vis", bufs=6,
                          name=f"vis{b}")
            nc.sync.dma_start(vis[:], vist_d[b])
            visq[b] = vis
            if FP8_WV:
                vis8 = vp.tile([128, 4, CB], FP8, tag="vis8", bufs=6,
                               name=f"vis8_{b}")
                nc.scalar.dma_start(vis8[:], vis8_d[b])
            vt = wp.tile([128, 4, CB], F16, tag="vt", bufs=3, name=f"vt{b}")
            vtq[b] = vt
            for half in range(2):
                ps = pmm.tile([128, 2, 512], F32, tag="mm")
                for i in range(2):
                    eo = half * 2 + i
                    if FP8_WV:
                        for dp_ in range(2):
                            nc.tensor.matmul(
                                ps[:, i, :CB],
                                wv8[:, 2 * dp_:2 * dp_ + 2,
                                    eo * 128:(eo + 1) * 128],
                                vis8[:, 2 * dp_:2 * dp_ + 2, :],
                                start=(dp_ == 0), stop=(dp_ == 1),
                                perf_mode=mybir.MatmulPerfMode.DoubleRow)
                    else:
                        for do in range(4):
                            nc.tensor.matmul(
                                ps[:, i, :CB],
                                wvt[:, do, eo * 128:(eo + 1) * 128],
                                vis[:, do, :],
                                start=(do == 0), stop=(do == 3))
                if has_bv:
                    for i in range(2):
                        eo = half * 2 + i
                        nc.scalar.activation(vt[:, eo, :], ps[:, i, :CB],
                                             AF.Relu, bias=bv[:, eo:eo + 1],
                                             scale=rscale)
                else:
                    nc.scalar.activation(vt[:, half * 2:half * 2 + 2, :],
                                         ps[:, :, :CB], AF.Relu, scale=rscale)

        def stage_b(b):
            vt = vtq.pop(b)
            psv = pvs.tile([64, 512], F32, tag="vs")
            for eo in range(4):
                nc.tensor.matmul(psv[:S, :CB], avt[:, eo, :],
                                 vt[:, eo, :], start=(eo == 0), stop=False)
            nc.tensor.matmul(psv[:S, :CB], ones[0:1, 0:S],
                             asr[0:1, b * CB:(b + 1) * CB],
                             start=False, stop=True)
            th = wp.tile([S, CB], F16, tag="th", bufs=3, name=f"th{b}")
            thq[b] = th
            nc.scalar.activation(th[:], psv[:S, :CB], AF.Tanh)

        def stage_c(b):
            th = thq.pop(b)
            vis = visq.pop(b)
            psf = pft.tile([1, 512], F32, tag="ft")
            nc.tensor.matmul(psf[0:1, :CB], aft[:], th[:],
                             start=True, stop=True)
            ex = wp.tile([1, CB], F16, tag="ex", bufs=3, name=f"ex{b}")
            nc.scalar.activation(ex[:], psf[0:1, :CB], AF.Exp)

            t7 = wp.tile([128, 5, CB], F16, tag="t7", bufs=3, name=f"t7_{b}")
            nc.gpsimd.partition_broadcast(t7[:, 4, :], ex[:])
            # t7[0:4] = visT * att (3 chunks on DVE, 1 on gpsimd)
            in0, in1 = broadcast_tensor_aps(vis[:, 0:3, :], t7[:, 4:5, :])
            nc.vector.tensor_tensor(t7[:, 0:3, :], in0, in1, ALU.mult)
            in0, in1 = broadcast_tensor_aps(vis[:, 3:4, :], t7[:, 4:5, :])
            nc.gpsimd.tensor_tensor(t7[:, 3:4, :], in0, in1, ALU.mult)

            # segmented sums over s for all 5 chunks on DVE;
            # outT chunk 4 = Z replicated per partition.
            nc.vector.reduce_sum(
                outT[:, :, b * NB:(b + 1) * NB],
                t7[:].rearrange("p f (n s) -> p f n s", s=S),
                axis=AX.X)
            nc.vector.reciprocal(rinv[0:1, b * NB:(b + 1) * NB],
                                 outT[0:1, 4, b * NB:(b + 1) * NB])

        for b in range(NBLK + 2):
            if b < NBLK:
                stage_a(b)
            if 1 <= b <= NBLK:
                stage_b(b - 1)
            if b >= 2:
                stage_c(b - 2)

        # ---------- epilogue: transpose out.T back, scale by 1/Z, store ----------
        for it in range(n_nt):
            n0 = it * 128
            nr = min(128, n_n - n0)
            psr = pvs.tile([128, 128], F32, tag="vs")
            nc.tensor.transpose(psr[:nr, 0:1], rinv[0:1, n0:n0 + nr],
                                idf[0:1, 0:1])
            rin = wp.tile([128, 1], F32, tag="rin")
            nc.vector.tensor_copy(rin[:nr, :], psr[:nr, 0:1])
            on = wp.tile([128, D], F32, tag="on")
            for do in range(4):
                pso = pmm.tile([128, 2, 512], F32, tag="mm")
                nc.tensor.transpose(pso[:nr, 0, :128], outT[:, do, n0:n0 + nr],
                                    idf[:, :])
                nc.scalar.activation(on[:nr, do * 128:(do + 1) * 128],
                                     pso[:nr, 0, :128], AF.Copy,
                                     scale=rin[:nr, 0:1])
            nc.sync.dma_start(out_d[n0:n0 + nr, :], on[:nr, :])

    nc.finalize()
    return nc


def prep_consts(Wa, ba, Wv, bv, Aa, Av, Af):
    import ml_dtypes
    f16 = np.float16
    f = np.float32
    c = {}
    # wvt[p, do, e] = Wv[e, do*128+p]
    wvt_host = np.ascontiguousarray(Wv.T.reshape(4, 128, E).transpose(1, 0, 2))
    if FP8_WV:
        c["Wv8"] = (wvt_host * WSCALE).astype(ml_dtypes.float8_e4m3)
    else:
        c["WvT"] = wvt_host.astype(f16)
    # wat[a, eo, m] = Wa[eo*128+m, a]
    c["WaT"] = np.ascontiguousarray(
        Wa.T.reshape(A, 4, 128)).astype(f16)
    # aat[p, eo, s] = Aa[s, eo*128+p]
    c["AaT"] = np.ascontiguousarray(
        Aa.T.reshape(4, 128, S).transpose(1, 0, 2)).astype(f16)
    c["AvT"] = np.ascontiguousarray(
        Av.T.reshape(4, 128, S).transpose(1, 0, 2)).astype(f16)
    c["AfT"] = np.ascontiguousarray(Af.reshape(1, S).T).astype(f16)
    c["ba_l"] = np.ascontiguousarray(ba.reshape(4, 128).T).astype(f)
    c["bv_l"] = np.ascontiguousarray(bv.reshape(4, 128).T).astype(f)
    c["ident"] = np.eye(128, dtype=f).astype(f16)
    c["identf"] = np.eye(128, dtype=f)
    c["ones"] = np.ones((1, 128), dtype=f16)
    return c


_CACHE = {}


def kernel(audio, visual, Wa, ba, Wv, bv, Aa, Av, Af):
    from concourse.bass_utils import run_bass_kernel_spmd
    import ml_dtypes
    f16 = np.float16

    audio = np.asarray(audio, np.float32)
    visual = np.asarray(visual, np.float32)
    ba = np.asarray(ba, np.float32)
    bv = np.asarray(bv, np.float32)
    n_n = (B // NCORES) * T  # 320

    has_ba = bool(np.any(ba))
    has_bv = bool(np.any(bv))
    key = ("nc", has_ba, has_bv)
    if key not in _CACHE:
        _CACHE[key] = build_module(n_n, has_ba, has_bv)
    nc = _CACHE[key]

    consts = prep_consts(np.asarray(Wa, np.float32), ba,
                         np.asarray(Wv, np.float32), bv,
                         np.asarray(Aa, np.float32),
                         np.asarray(Av, np.float32),
                         np.asarray(Af, np.float32))
    bs = B // NCORES
    vis_16 = visual.astype(f16)          # one cast for the full tensor
    aud_16 = audio.astype(f16)
    in_maps = []
    for c in range(NCORES):
        m = dict(consts)
        m["audio"] = np.ascontiguousarray(
            aud_16[c * bs:(c + 1) * bs].reshape(n_n, A))
        # visT[b, p, do, j] = vis[b*CB + j, do*128 + p]
        v = vis_16[c * bs:(c + 1) * bs].reshape(NBLK, CB, 4, 128)
        vT = np.ascontiguousarray(v.transpose(0, 3, 2, 1))
        m["visT"] = vT
        if FP8_WV:
            m["visT8"] = vT.astype(ml_dtypes.float8_e4m3)
        in_maps.append(m)

    res = run_bass_kernel_spmd(nc, in_maps, core_ids=list(range(NCORES)))
    _CACHE["last_res"] = res
    out = np.concatenate(
        [r["out"].reshape(bs, T, D) for r in res.results], axis=0)
    return out.astype(np.float32)


# revision 20
# speedup vs baseline: 1.4789x; 1.0578x over previous
"""Trainium2 Bass kernel for nn_AttentionNet (audio-visual attention).

Data-parallel across 8 NeuronCores: batch B=256 split 32 per core, i.e.
n_n = 320 (b,t) rows and 320*49 = 15680 visual rows per core.

Per-core math (n indexes the 320 rows, s in [0,49), d/e in [0,512)):
    a_t = relu(audio @ Wa.T + ba)            [N,512]
    v_t = relu(vis @ Wv.T + bv)              [N,49,512]
    a_s = a_t @ Aa.T                         [N,49]
    v_s = v_t @ Av.T                         [N,49,49]
    f   = (tanh(a_s[:,:,None] + v_s)) @ Af.T [N,49]
    att = softmax_s(f)
    out = att @ vis                          [N,512]

Design notes:
- visual is pre-transposed ON THE HOST into a blocked layout
  visT[b, p, do, j] = vis[b*490 + j, do*128 + p] (fp16 for the epilogue
  weighted sum, fp8e4 copy for the Wv matmul), so the device never
  transposes the 32 MB visual tensor.
- The dominant Wv matmul runs in fp8e4 with DoubleRow perf mode
  (2 k-tiles of 128 per matmul).  Weights are pre-scaled by 32 on the
  host; the relu activation's free scale undoes it.
- The softmax runs unnormalized; 1/Z is folded into the final output
  transpose as a per-partition scale.
- The attention-weighted sum over s uses a masked tensor_tensor_scan
  (state = mask*state + t7) so segment sums appear at s=48 of each
  49-column group; fp16 intermediates keep the downcast noise at ~5e-4.
- The broadcast of the attention row across partitions and the Z
  (softmax denominator) path run on the otherwise-idle GPSIMD engine.
"""

import numpy as np

try:
    import concourse.bass as bass
except ImportError:
    import sys as _sys
    for _p in ("/opt/trn_rl_repo", "/root/.axon_site/_ro/trn_rl_repo"):
        if _p not in _sys.path:
            _sys.path.insert(0, _p)
    import concourse.bass as bass
import concourse.mybir as mybir
import concourse.tile as tile
from concourse import bacc
from concourse.bass import broadcast_tensor_aps

F32 = mybir.dt.float32
F16 = mybir.dt.float16
FP8 = mybir.dt.float8e4
AX = mybir.AxisListType
ALU = mybir.AluOpType
AF = mybir.ActivationFunctionType

NCORES = 8
B, T, S, D, E, A = 256, 10, 49, 512, 512, 128
NB = 10              # n's per column block
CB = NB * S          # 490 columns per block
NBLK = (B // NCORES) * T // NB   # 32 blocks per core

FP8_WV = True        # run the Wv matmul in fp8e4 with DoubleRow
WSCALE = 32.0        # host-side weight scale for fp8 dynamic range


def build_module(n_n, has_ba=False, has_bv=False):
    """Build the Bass module for one core handling n_n (b,t) rows."""
    assert n_n == NB * NBLK
    rows = n_n * S
    n_nt = (n_n + 127) // 128            # 128-row n tiles (a-path / epilogue)

    nc = bacc.Bacc("TRN2", debug=False)

    vist_d = nc.dram_tensor("visT", [NBLK, 128, 4, CB], F16,
                            kind="ExternalInput").ap()
    if FP8_WV:
        vis8_d = nc.dram_tensor("visT8", [NBLK, 128, 4, CB], FP8,
                                kind="ExternalInput").ap()
        wv8_d = nc.dram_tensor("Wv8", [128, 4, E], FP8,
                               kind="ExternalInput").ap()
    else:
        wvt_d = nc.dram_tensor("WvT", [128, 4, E], F16,
                               kind="ExternalInput").ap()
    aud_d = nc.dram_tensor("audio", [n_n, A], F16, kind="ExternalInput").ap()
    wat_d = nc.dram_tensor("WaT", [128, 4, 128], F16, kind="ExternalInput").ap()
    aat_d = nc.dram_tensor("AaT", [128, 4, S], F16, kind="ExternalInput").ap()
    avt_d = nc.dram_tensor("AvT", [128, 4, S], F16, kind="ExternalInput").ap()
    aft_d = nc.dram_tensor("AfT", [S, 1], F16, kind="ExternalInput").ap()
    bal_d = nc.dram_tensor("ba_l", [128, 4], F32, kind="ExternalInput").ap()
    bvl_d = nc.dram_tensor("bv_l", [128, 4], F32, kind="ExternalInput").ap()
    idn_d = nc.dram_tensor("ident", [128, 128], F16, kind="ExternalInput").ap()
    idf_d = nc.dram_tensor("identf", [128, 128], F32, kind="ExternalInput").ap()
    one_d = nc.dram_tensor("ones", [1, 128], F16, kind="ExternalInput").ap()
    out_d = nc.dram_tensor("out", [n_n, D], F32, kind="ExternalOutput").ap()

    with tile.TileContext(nc) as tc, \
         tc.tile_pool(name="consts", bufs=1) as cp, \
         tc.tile_pool(name="vload", bufs=4) as vp, \
         tc.tile_pool(name="work", bufs=2) as wp, \
         tc.tile_pool(name="dram", bufs=1, space="DRAM") as dp, \
         tc.tile_pool(name="ps_mm", bufs=2, space="PSUM") as pmm, \
         tc.tile_pool(name="ps_vs", bufs=2, space="PSUM") as pvs, \
         tc.tile_pool(name="ps_ft", bufs=2, space="PSUM") as pft:

        # ---------- constants ----------
        if FP8_WV:
            wv8 = cp.tile([128, 4, E], FP8, tag="wv8")
            nc.sync.dma_start(wv8[:], wv8_d)
        else:
            wvt = cp.tile([128, 4, E], F16, tag="wvt")
            nc.sync.dma_start(wvt[:], wvt_d)
        wat = cp.tile([128, 4, 128], F16, tag="wat")
        nc.sync.dma_start(wat[:], wat_d)
        aat = cp.tile([128, 4, S], F16, tag="aat")
        nc.sync.dma_start(aat[:], aat_d)
        avt = cp.tile([128, 4, S], F16, tag="avt")
        nc.sync.dma_start(avt[:], avt_d)
        aft = cp.tile([S, 1], F16, tag="aft")
        nc.sync.dma_start(aft[:], aft_d)
        ba = cp.tile([128, 4], F32, tag="ba")
        nc.sync.dma_start(ba[:], bal_d)
        bv = cp.tile([128, 4], F32, tag="bv")
        nc.sync.dma_start(bv[:], bvl_d)
        idn = cp.tile([128, 128], F16, tag="idn")
        nc.sync.dma_start(idn[:], idn_d)
        idf = cp.tile([128, 128], F32, tag="idf")
        nc.sync.dma_start(idf[:], idf_d)
        ones = cp.tile([1, 128], F16, tag="ones")
        nc.sync.dma_start(ones[:], one_d)

        audT = cp.tile([128, n_n], F16, tag="audT")     # audio.T  [a, n]
        atT = cp.tile([128, 4, n_n], F16, tag="atT")    # a_t.T    [e, n]
        asr = cp.tile([1, rows], F16, tag="asr")        # a_s row  [(n,s)]
        rinv = cp.tile([1, n_n], F32, tag="rinv")       # 1/Z per n
        # out.T [d, n] in chunks 0..3; chunk 4 row 0 = Z per n
        outT = cp.tile([128, 5, n_n], F32, tag="outT")

        # ---------- a-path prologue ----------
        for it in range(n_nt):
            n0 = it * 128
            nr = min(128, n_n - n0)
            an = wp.tile([128, A], F16, tag="an")
            nc.sync.dma_start(an[:nr, :], aud_d[n0:n0 + nr, :])
            ps = pvs.tile([128, 128], F16, tag="vs")
            nc.tensor.transpose(ps[:, :nr], an[:nr, :], idn[:nr, :nr])
            nc.vector.tensor_copy(audT[:, n0:n0 + nr], ps[:, :nr])

        for half in range(2):
            ps = pmm.tile([128, 2, 512], F32, tag="mm")
            for i in range(2):
                eo = half * 2 + i
                nc.tensor.matmul(ps[:, i, :n_n], wat[:, eo, :], audT[:],
                                 start=True, stop=True)
            if has_ba:
                for i in range(2):
                    eo = half * 2 + i
                    nc.scalar.activation(atT[:, eo, :], ps[:, i, :n_n],
                                         AF.Relu, bias=ba[:, eo:eo + 1])
            else:
                nc.scalar.activation(atT[:, half * 2:half * 2 + 2, :],
                                     ps[:, :, :n_n], AF.Relu)

        as_dram = dp.tile([1, rows], F16, tag="asd")
        for it in range(n_nt):
            n0 = it * 128
            nr = min(128, n_n - n0)
            psa = pvs.tile([128, 64], F32, tag="vs")
            for eo in range(4):
                nc.tensor.matmul(psa[:nr, :S], atT[:, eo, n0:n0 + nr],
                                 aat[:, eo, :],
                                 start=(eo == 0), stop=(eo == 3))
            asn = wp.tile([128, S], F16, tag="asn")
            nc.vector.tensor_copy(asn[:nr, :], psa[:nr, :S])
            dst = as_dram[0:1, n0 * S:(n0 + nr) * S]
            nc.sync.dma_start(dst.rearrange("one (n s) -> (one n) s", s=S),
                              asn[:nr, :])
        nc.sync.dma_start(asr[:], as_dram[:])

        # ---------- main loop (software-pipelined, 2-block skew) ----------
        # stage A(b): DMA + Wv matmul + relu          -> vt[b]
        # stage B(b): v_s matmul + a_s add + tanh     -> th[b]
        # stage C(b): f matmul, exp, broadcast, att*vis, segment-reduce
        # Consecutive instructions on each engine depend only on work from
        # previous blocks, so no engine head-of-line blocks on same-block
        # results.
        rscale = (1.0 / WSCALE) if FP8_WV else 1.0
        visq, vtq, thq = {}, {}, {}

        def stage_a(b):
            vis = vp.tile([128, 4, CB], F16, tag="vis", bufs=6,
                          name=f"vis{b}")
            nc.sync.dma_start(vis[:], vist_d[b])
            visq[b] = vis
            if FP8_WV:
                vis8 = vp.tile([128, 4, CB], FP8, tag="vis8", bufs=6,
                               name=f"vis8_{b}")
                nc.scalar.dma_start(vis8[:], vis8_d[b])
            vt = wp.tile([128, 4, CB], F16, tag="vt", bufs=3, name=f"vt{b}")
            vtq[b] = vt
            for half in range(2):
                ps = pmm.tile([128, 2, 512], F32, tag="mm")
                for i in range(2):
                    eo = half * 2 + i
                    if FP8_WV:
                        for dp_ in range(2):
                            nc.tensor.matmul(
                                ps[:, i, :CB],
                                wv8[:, 2 * dp_:2 * dp_ + 2,
                                    eo * 128:(eo + 1) * 128],
                                vis8[:, 2 * dp_:2 * dp_ + 2, :],
                                start=(dp_ == 0), stop=(dp_ == 1),
                                perf_mode=mybir.MatmulPerfMode.DoubleRow)
                    else:
                        for do in range(4):
                            nc.tensor.matmul(
                                ps[:, i, :CB],
                                wvt[:, do, eo * 128:(eo + 1) * 128],
                                vis[:, do, :],
                                start=(do == 0), stop=(do == 3))
                if has_bv:
                    for i in range(2):
                        eo = half * 2 + i
                        nc.scalar.activation(vt[:, eo, :], ps[:, i, :CB],
                                             AF.Relu, bias=bv[:, eo:eo + 1],
                                             scale=rscale)
                else:
                    nc.scalar.activation(vt[:, half * 2:half * 2 + 2, :],
                                         ps[:, :, :CB], AF.Relu, scale=rscale)

        def stage_b(b):
            vt = vtq.pop(b)
            psv = pvs.tile([64, 512], F32, tag="vs")
            for eo in range(4):
                nc.tensor.matmul(psv[:S, :CB], avt[:, eo, :],
                                 vt[:, eo, :], start=(eo == 0), stop=False)
            nc.tensor.matmul(psv[:S, :CB], ones[0:1, 0:S],
                             asr[0:1, b * CB:(b + 1) * CB],
                             start=False, stop=True)
            th = wp.tile([S, CB], F16, tag="th", bufs=3, name=f"th{b}")
            thq[b] = th
            nc.scalar.activation(th[:], psv[:S, :CB], AF.Tanh)

        t7q = {}

        def stage_c1(b):
            th = thq.pop(b)
            vis = visq.pop(b)
            psf = pft.tile([1, 512], F32, tag="ft")
            nc.tensor.matmul(psf[0:1, :CB], aft[:], th[:],
                             start=True, stop=True)
            ex = wp.tile([1, CB], F16, tag="ex", bufs=3, name=f"ex{b}")
            nc.scalar.activation(ex[:], psf[0:1, :CB], AF.Exp)

            t7 = wp.tile([128, 5, CB], F16, tag="t7", bufs=3, name=f"t7_{b}")
            t7q[b] = t7
            nc.gpsimd.partition_broadcast(t7[:, 4, :], ex[:])
            # t7[0:4] = visT * att (3 chunks on DVE, 1 on gpsimd)
            in0, in1 = broadcast_tensor_aps(vis[:, 0:3, :], t7[:, 4:5, :])
            nc.vector.tensor_tensor(t7[:, 0:3, :], in0, in1, ALU.mult)
            in0, in1 = broadcast_tensor_aps(vis[:, 3:4, :], t7[:, 4:5, :])
            nc.gpsimd.tensor_tensor(t7[:, 3:4, :], in0, in1, ALU.mult)

        def stage_c2(b):
            t7 = t7q.pop(b)
            t7v = t7[:].rearrange("p f (n s) -> p f n s", s=S)
            # fold pairs: t8[..,k] = t7[..,k] + t7[..,25+k] (k<24); t8[..,24]=t7[..,24]
            t8 = wp.tile([128, 5, NB, 25], F16, tag="t8", bufs=2,
                         name=f"t8_{b}")
            nc.vector.tensor_tensor(t8[:, :, :, 0:24], t7v[:, :, :, 0:24],
                                    t7v[:, :, :, 25:49], ALU.add)
            nc.vector.tensor_copy(t8[:, :, :, 24:25], t7v[:, :, :, 24:25])
            # segmented sums over s for all 5 chunks on DVE;
            # outT chunk 4 = Z replicated per partition.
            nc.vector.reduce_sum(
                outT[:, :, b * NB:(b + 1) * NB], t8[:], axis=AX.X)
            nc.vector.reciprocal(rinv[0:1, b * NB:(b + 1) * NB],
                                 outT[0:1, 4, b * NB:(b + 1) * NB])

        for b in range(NBLK + 3):
            if b < NBLK:
                stage_a(b)
            if 1 <= b <= NBLK:
                stage_b(b - 1)
            if 2 <= b <= NBLK + 1:
                stage_c1(b - 2)
            if b >= 3:
                stage_c2(b - 3)

        # ---------- epilogue: transpose out.T back, scale by 1/Z, store ----------
        for it in range(n_nt):
            n0 = it * 128
            nr = min(128, n_n - n0)
            psr = pvs.tile([128, 128], F32, tag="vs")
            nc.tensor.transpose(psr[:nr, 0:1], rinv[0:1, n0:n0 + nr],
                                idf[0:1, 0:1])
            rin = wp.tile([128, 1], F32, tag="rin")
            nc.vector.tensor_copy(rin[:nr, :], psr[:nr, 0:1])
            on = wp.tile([128, D], F32, tag="on")
            for do in range(4):
                pso = pmm.tile([128, 2, 512], F32, tag="mm")
                nc.tensor.transpose(pso[:nr, 0, :128], outT[:, do, n0:n0 + nr],
                                    idf[:, :])
                nc.scalar.activation(on[:nr, do * 128:(do + 1) * 128],
                                     pso[:nr, 0, :128], AF.Copy,
                                     scale=rin[:nr, 0:1])
            nc.sync.dma_start(out_d[n0:n0 + nr, :], on[:nr, :])

    nc.finalize()
    return nc


def prep_consts(Wa, ba, Wv, bv, Aa, Av, Af):
    import ml_dtypes
    f16 = np.float16
    f = np.float32
    c = {}
    # wvt[p, do, e] = Wv[e, do*128+p]
    wvt_host = np.ascontiguousarray(Wv.T.reshape(4, 128, E).transpose(1, 0, 2))
    if FP8_WV:
        c["Wv8"] = (wvt_host * WSCALE).astype(ml_dtypes.float8_e4m3)
    else:
        c["WvT"] = wvt_host.astype(f16)
    # wat[a, eo, m] = Wa[eo*128+m, a]
    c["WaT"] = np.ascontiguousarray(
        Wa.T.reshape(A, 4, 128)).astype(f16)
    # aat[p, eo, s] = Aa[s, eo*128+p]
    c["AaT"] = np.ascontiguousarray(
        Aa.T.reshape(4, 128, S).transpose(1, 0, 2)).astype(f16)
    c["AvT"] = np.ascontiguousarray(
        Av.T.reshape(4, 128, S).transpose(1, 0, 2)).astype(f16)
    c["AfT"] = np.ascontiguousarray(Af.reshape(1, S).T).astype(f16)
    c["ba_l"] = np.ascontiguousarray(ba.reshape(4, 128).T).astype(f)
    c["bv_l"] = np.ascontiguousarray(bv.reshape(4, 128).T).astype(f)
    c["ident"] = np.eye(128, dtype=f).astype(f16)
    c["identf"] = np.eye(128, dtype=f)
    c["ones"] = np.ones((1, 128), dtype=f16)
    return c


_CACHE = {}


def kernel(audio, visual, Wa, ba, Wv, bv, Aa, Av, Af):
    from concourse.bass_utils import run_bass_kernel_spmd
    import ml_dtypes
    f16 = np.float16

    audio = np.asarray(audio, np.float32)
    visual = np.asarray(visual, np.float32)
    ba = np.asarray(ba, np.float32)
    bv = np.asarray(bv, np.float32)
    n_n = (B // NCORES) * T  # 320

    has_ba = bool(np.any(ba))
    has_bv = bool(np.any(bv))
    key = ("nc", has_ba, has_bv)
    if key not in _CACHE:
        _CACHE[key] = build_module(n_n, has_ba, has_bv)
    nc = _CACHE[key]

    consts = prep_consts(np.asarray(Wa, np.float32), ba,
                         np.asarray(Wv, np.float32), bv,
                         np.asarray(Aa, np.float32),
                         np.asarray(Av, np.float32),
                         np.asarray(Af, np.float32))
    bs = B // NCORES
    vis_16 = visual.astype(f16)          # one cast for the full tensor
    aud_16 = audio.astype(f16)
    in_maps = []
    for c in range(NCORES):
        m = dict(consts)
        m["audio"] = np.ascontiguousarray(
            aud_16[c * bs:(c + 1) * bs].reshape(n_n, A))
        # visT[b, p, do, j] = vis[b*CB + j, do*128 + p]
        v = vis_16[c * bs:(c + 1) * bs].reshape(NBLK, CB, 4, 128)
        vT = np.ascontiguousarray(v.transpose(0, 3, 2, 1))
        m["visT"] = vT
        if FP8_WV:
            m["visT8"] = vT.astype(ml_dtypes.float8_e4m3)
        in_maps.append(m)

    res = run_bass_kernel_spmd(nc, in_maps, core_ids=list(range(NCORES)))
    _CACHE["last_res"] = res
    out = np.concatenate(
        [r["out"].reshape(bs, T, D) for r in res.results], axis=0)
    return out.astype(np.float32)
